# revision 7
# baseline (speedup 1.0000x reference)
# ContextRCNN attention-bias kernel for 8 Trainium2 NeuronCores.
#
# Reference computation (see problem statement):
#   central:[N,C,7,7] -> mean-pool -> Q-MLP -> l2norm -> queries [N,QK]
#   context:[T,C,7,7] -> mean-pool -> K/V-MLPs (K l2normed)
#   softmax(Q @ K^T * 6.25) @ V -> final MLP -> [N, C]
#
# Distribution (one SPMD NEFF on 8 cores, rank == q-shard == t-shard index):
#   - central rows sharded N/8=512 per core; context rows sharded T/8=1536.
#   - Each core pools+MLPs only its own shards (so the 822MB of input is read
#     exactly once across the chip), computes local queries, AllGathers the
#     (small, 4MB) query matrix, then computes the partial attention of ALL
#     4096 queries against its LOCAL 1536 keys/values:  numer_r = exp(S_r)^T
#     @ [V_r | 1].  A ReduceScatter sums the partials over cores and hands
#     rank r exactly its q-shard rows; divide by the gathered denominator
#     column and run the final MLP locally.
#   - Softmax needs no max-subtraction: logits are 6.25*cosine in [-6.25,6.25].
#
# Layout trick: "transposed" first MLP layers (lhsT = weight in its natural
# [in,out] layout, rhs = pooled^T) followed by natural second layers flip the
# orientation back, so no transposes are needed on the big T pipeline; only
# the small pooled/Q/attn matrices get PE transposes.

import numpy as np
from contextlib import ExitStack

import concourse.bass as bass
import concourse.mybir as mybir
import concourse.tile as tile
from concourse import bacc
from concourse.bass_utils import run_bass_kernel_spmd
from concourse.masks import make_identity

AF = mybir.ActivationFunctionType
DT = mybir.dt.float32

M = 8                    # cores
N, T, C, S = 4096, 12288, 256, 7
NS, TS = N // M, T // M  # 512 q rows / 1536 kv rows per core
H = 512                  # MLP hidden
D = 256                  # QK == VD == C
SS = S * S               # 49
SCALE = 1.0 / (0.01 * C ** 0.5)   # 6.25
NT_Q = NS // 128         # 4  q-tiles per core
NT_T = TS // 128         # 12 t-tiles per core
NCOL = D + 1             # V plus ones column -> denominator for free
CSPLIT = 4               # channel split for the raw pooling loads
CCH = C // CSPLIT        # channels per load
RAWF = CCH * SS          # floats per partition row per load

# float32r matmul operands stream at 1 cycle/row (4x faster than float32)
# with a reduced-precision multiply. The BIR verifier requires every SBUF
# tensor consumed by an fp32r matmul to be *written* as float32r (so values
# are pre-rounded), so the operand tiles/DRAM tensors themselves are declared
# float32r; numpy still sees plain fp32 bytes.


def build_nc(fast_mm=False, fast_mlp=False):
    nc = bacc.Bacc("TRN2", target_bir_lowering=False, debug=False, num_devices=M)
    adt = mybir.dt.float32r if fast_mm else DT    # attention matmul operands
    mdt = mybir.dt.float32r if fast_mlp else DT   # MLP matmul operands

    central = nc.dram_tensor("central_sh", [NS, C, S, S], DT, kind="ExternalInput")
    context = nc.dram_tensor("context_sh", [TS, C, S, S], DT, kind="ExternalInput")
    wnames = ["qw1", "qw2", "kw1", "kw2", "vw1", "vw2", "fw1", "fw2"]
    wshapes = {"1": [C, H], "2": [H, D]}
    wdram = {n: nc.dram_tensor(n, wshapes[n[-1]], mdt, kind="ExternalInput")
             for n in wnames}
    out_sh = nc.dram_tensor("out_sh", [NS, C], DT, kind="ExternalOutput")

    qt_in = nc.dram_tensor("qt_in", [D, NS], adt)                      # AG input
    qt_out = nc.dram_tensor("qt_out", [M * D, NS], adt, addr_space="Shared")
    rs_in = nc.dram_tensor("rs_in", [N, NCOL], DT)                     # RS input
    rs_out = nc.dram_tensor("rs_out", [NS, NCOL], DT)

    with tile.TileContext(nc) as tc, ExitStack() as ctx:
        ident_pool = ctx.enter_context(tc.tile_pool(name="ident", bufs=1))
        ident = ident_pool.tile([128, 128], DT)
        make_identity(nc, ident[:])

        # SBUF tensors that live across phases
        kvq_pool = ctx.enter_context(tc.tile_pool(name="kvq", bufs=1))
        ones_col = kvq_pool.tile([128, 1], DT, tag="ones", name="ones")
        nc.gpsimd.memset(ones_col[:], 1.0)
        kt_sb = [kvq_pool.tile([128, TS], DT, tag=f"kt{i}") for i in range(2)]
        vo_sb = [kvq_pool.tile([128, NCOL], DT, tag=f"vo{i}") for i in range(NT_T)]
        qt_all = [kvq_pool.tile([128, NS], DT, tag=f"qta{i}") for i in range(2 * M)]

        def transpose128(dst_sb, src_sb, tp_pool):
            """dst[128,128] = src[128,128]^T via PE."""
            ps = tp_pool.tile([128, 128], DT)
            nc.tensor.transpose(ps[:], src_sb, ident[:])
            nc.vector.tensor_copy(dst_sb, ps[:])

        def load_w(pool, name):
            """Load a weight matrix as a list of 128-row SBUF tiles."""
            shape = wshapes[name[-1]]
            tiles = []
            for i in range(shape[0] // 128):
                t = pool.tile([128, shape[1]], DT, tag=f"{name}_{i}")
                nc.sync.dma_start(t[:], wdram[name].ap()[i * 128:(i + 1) * 128, :])
                tiles.append(t)
            return tiles

        def pool_rows(dram_t, row0, raw_pool, pooled_pool):
            """Sum-pool 128 rows of [rows,C,7,7] -> pooled [128, C] tile.
            (The 1/49 mean scale cancels in l2norm for Q/K and is folded
            into vw2 on the host for V.)"""
            pooled = pooled_pool.tile([128, C], DT)
            src = dram_t.ap().rearrange("t c h w -> t (c h w)")
            for cs in range(CSPLIT):
                raw = raw_pool.tile([128, RAWF], DT, tag="raw")
                nc.sync.dma_start(
                    raw[:],
                    src[row0:row0 + 128, cs * RAWF:(cs + 1) * RAWF])
                nc.vector.reduce_sum(
                    pooled[:, cs * CCH:(cs + 1) * CCH],
                    raw[:].rearrange("p (c s) -> p c s", s=SS),
                    axis=mybir.AxisListType.X)
            return pooled

        def mlp_l1_T(w1_tiles, xT, out_tiles, nfree, ps_pool, fast):
            """hidden^T[h,n] = relu(w1^T @ x^T): out_tiles = 4 x [128, nfree]."""
            for ht in range(H // 128):
                for ch in range(nfree // 512):
                    ps = ps_pool.tile([128, 512], DT, tag="mm512")
                    for ck in range(C // 128):
                        nc.tensor.matmul(
                            ps[:],
                            w1_tiles[ck][:, ht * 128:(ht + 1) * 128],
                            xT[ck][:, ch * 512:(ch + 1) * 512],
                            start=(ck == 0), stop=(ck == 1))
                    nc.scalar.activation(
                        out_tiles[ht][:, ch * 512:(ch + 1) * 512], ps[:], AF.Relu)

        def mlp_l2_nat(hid_tiles, w2_tiles, nt, ps_pool, fast):
            """x[n,d] psum tile = hidden @ w2 for 128-row block nt."""
            ps = ps_pool.tile([128, D], DT, tag="mm256")
            for hk in range(H // 128):
                nc.tensor.matmul(
                    ps[:],
                    hid_tiles[hk][:, nt * 128:(nt + 1) * 128],
                    w2_tiles[hk][:],
                    start=(hk == 0), stop=(hk == 3))
            return ps

        # ---------------- Phase A: pooling, Q path, K/V path ----------------
        with tc.tile_pool(name="raw", bufs=2) as raw_pool, \
             tc.tile_pool(name="pooled", bufs=2) as pooled_pool, \
             tc.tile_pool(name="pooledT", bufs=1) as pt_pool, \
             tc.tile_pool(name="wA", bufs=1) as wA_pool, \
             tc.tile_pool(name="hid", bufs=1) as hid_pool, \
             tc.tile_pool(name="small", bufs=2) as small_pool, \
             tc.tile_pool(name="psA", bufs=2, space="PSUM") as psA, \
             tc.tile_pool(name="psT", bufs=2, space="PSUM") as psT:

            qw1 = load_w(wA_pool, "qw1"); qw2 = load_w(wA_pool, "qw2")
            kw1 = load_w(wA_pool, "kw1"); kw2 = load_w(wA_pool, "kw2")
            vw1 = load_w(wA_pool, "vw1"); vw2 = load_w(wA_pool, "vw2")

            # --- central: pool + transpose ---
            cenT = [pt_pool.tile([128, NS], DT, tag=f"cenT{i}") for i in range(2)]
            for ntile in range(NT_Q):
                pooled = pool_rows(central, ntile * 128, raw_pool, pooled_pool)
                for ct in range(2):
                    transpose128(cenT[ct][:, ntile * 128:(ntile + 1) * 128],
                                 pooled[:, ct * 128:(ct + 1) * 128], psT)

            # --- Q MLP (transposed l1, natural l2), l2norm * 6.25, transpose ---
            hq = [hid_pool.tile([128, NS], DT, tag=f"hid{i}") for i in range(4)]
            mlp_l1_T(qw1, cenT, hq, NS, psA, fast_mlp)
            qt_sh = [wA_pool.tile([128, NS], DT, tag=f"qtsh{i}") for i in range(2)]
            for ntile in range(NT_Q):
                q_ps = mlp_l2_nat(hq, qw2, ntile, psA, fast_mlp)
                sq = small_pool.tile([128, D], DT, tag="sq")
                nc.scalar.activation(sq[:], q_ps[:], AF.Square)
                ssq = small_pool.tile([128, 1], DT, tag="ssq")
                nc.vector.reduce_sum(ssq[:], sq[:], axis=mybir.AxisListType.X)
                nrm = small_pool.tile([128, 1], DT, tag="nrm")
                nc.scalar.activation(nrm[:], ssq[:], AF.Sqrt)
                rcp = small_pool.tile([128, 1], DT, tag="rcp")
                nc.vector.reciprocal(rcp[:], nrm[:])
                qn = small_pool.tile([128, D], DT, tag="qn")
                nc.vector.tensor_scalar(
                    qn[:], q_ps[:], rcp[:], SCALE,
                    op0=mybir.AluOpType.mult, op1=mybir.AluOpType.mult)
                for ct in range(2):
                    transpose128(qt_sh[ct][:, ntile * 128:(ntile + 1) * 128],
                                 qn[:, ct * 128:(ct + 1) * 128], psT)
            for ct in range(2):
                nc.sync.dma_start(qt_in.ap()[ct * 128:(ct + 1) * 128, :], qt_sh[ct][:])
            nc.gpsimd.collective_compute(
                "AllGather", mybir.AluOpType.bypass,
                replica_groups=[list(range(M))],
                ins=[qt_in[:]], outs=[qt_out[:]])

            # --- context: pool + transpose ---
            ctxT = [pt_pool.tile([128, TS], DT, tag=f"ctxT{i}") for i in range(2)]
            for ttile in range(NT_T):
                pooled = pool_rows(context, ttile * 128, raw_pool, pooled_pool)
                for ct in range(2):
                    transpose128(ctxT[ct][:, ttile * 128:(ttile + 1) * 128],
                                 pooled[:, ct * 128:(ct + 1) * 128], psT)

            # --- K path: transposed l1, natural l2, l2norm, transpose -> KT ---
            hk = [hid_pool.tile([128, TS], DT, tag=f"hid{i}") for i in range(4)]
            mlp_l1_T(kw1, ctxT, hk, TS, psA, fast_mlp)
            for ttile in range(NT_T):
                k_ps = mlp_l2_nat(hk, kw2, ttile, psA, fast_mlp)
                sq = small_pool.tile([128, D], DT, tag="sq")
                nc.scalar.activation(sq[:], k_ps[:], AF.Square)
                ssq = small_pool.tile([128, 1], DT, tag="ssq")
                nc.vector.reduce_sum(ssq[:], sq[:], axis=mybir.AxisListType.X)
                nrm = small_pool.tile([128, 1], DT, tag="nrm")
                nc.scalar.activation(nrm[:], ssq[:], AF.Sqrt)
                rcp = small_pool.tile([128, 1], DT, tag="rcp")
                nc.vector.reciprocal(rcp[:], nrm[:])
                kn = small_pool.tile([128, D], DT, tag="qn")
                nc.vector.tensor_scalar_mul(kn[:], k_ps[:], rcp[:])
                for ct in range(2):
                    transpose128(kt_sb[ct][:, ttile * 128:(ttile + 1) * 128],
                                 kn[:, ct * 128:(ct + 1) * 128], psT)

            # --- V path: transposed l1, natural l2 -> [V | 1] tiles ---
            hv = [hid_pool.tile([128, TS], DT, tag=f"hid{i}") for i in range(4)]
            mlp_l1_T(vw1, ctxT, hv, TS, psA, fast_mlp)
            for ttile in range(NT_T):
                v_ps = mlp_l2_nat(hv, vw2, ttile, psA, fast_mlp)
                nc.vector.tensor_copy(vo_sb[ttile][:, 0:D], v_ps[:])
                nc.vector.tensor_copy(vo_sb[ttile][:, D:NCOL], ones_col[:])

        # load gathered queries to SBUF
        for i in range(2 * M):
            nc.sync.dma_start(qt_all[i][:], qt_out.ap()[i * 128:(i + 1) * 128, :])

        # ---------------- Phase B: attention over local t ----------------
        with tc.tile_pool(name="est", bufs=3) as e_pool, \
             tc.tile_pool(name="nmcp", bufs=4) as nm_pool, \
             tc.tile_pool(name="psST", bufs=2, space="PSUM") as ps_st, \
             tc.tile_pool(name="psNM", bufs=4, space="PSUM") as ps_nm:
            for qb in range(M):
                nm_ps = [ps_nm.tile([128, NCOL], DT, tag="nm") for _ in range(NT_Q)]
                prev_e = None
                for tt in range(NT_T):
                    st = ps_st.tile([128, 512], DT, tag="st")
                    for dk in range(2):
                        nc.tensor.matmul(
                            st[:],
                            kt_sb[dk][:, tt * 128:(tt + 1) * 128],
                            qt_all[2 * qb + dk][:],
                            start=(dk == 0), stop=(dk == 1))
                    e_sb = e_pool.tile([128, 512], adt, tag="e")
                    nc.scalar.activation(e_sb[:], st[:], AF.Exp)
                    if prev_e is not None:
                        ptt = tt - 1
                        for qt in range(NT_Q):
                            nc.tensor.matmul(
                                nm_ps[qt][:],
                                prev_e[:, qt * 128:(qt + 1) * 128],
                                vo_sb[ptt][:],
                                start=(ptt == 0), stop=False,
                                skip_group_check=True)
                    prev_e = e_sb
                for qt in range(NT_Q):
                    nc.tensor.matmul(
                        nm_ps[qt][:],
                        prev_e[:, qt * 128:(qt + 1) * 128],
                        vo_sb[NT_T - 1][:],
                        start=False, stop=True, skip_group_check=True)
                for qt in range(NT_Q):
                    nm_sb = nm_pool.tile([128, NCOL], DT, tag="nmsb")
                    nc.vector.tensor_copy(nm_sb[:], nm_ps[qt][:])
                    r0 = qb * NS + qt * 128
                    nc.sync.dma_start(rs_in.ap()[r0:r0 + 128, :], nm_sb[:])

        nc.gpsimd.collective_compute(
            "ReduceScatter", mybir.AluOpType.add,
            replica_groups=[list(range(M))],
            ins=[rs_in[:]], outs=[rs_out[:]])

        # ---------------- Phase C: divide + final MLP ----------------
        with tc.tile_pool(name="fin", bufs=2) as fin_pool, \
             tc.tile_pool(name="wC", bufs=1) as wC_pool, \
             tc.tile_pool(name="hidC", bufs=4) as hidC_pool, \
             tc.tile_pool(name="psC", bufs=2, space="PSUM") as psC, \
             tc.tile_pool(name="psTC", bufs=2, space="PSUM") as psTC:
            fw1 = load_w(wC_pool, "fw1"); fw2 = load_w(wC_pool, "fw2")
            attnT = [fin_pool.tile([128, NS], DT, tag=f"attnT{i}") for i in range(2)]
            for ntile in range(NT_Q):
                att = fin_pool.tile([128, NCOL], DT, tag="att")
                nc.sync.dma_start(
                    att[:], rs_out.ap()[ntile * 128:(ntile + 1) * 128, :])
                rd = fin_pool.tile([128, 1], DT, tag="rd")
                nc.vector.reciprocal(rd[:], att[:, D:NCOL])
                an = fin_pool.tile([128, D], DT, tag="an")
                nc.vector.tensor_scalar_mul(an[:], att[:, 0:D], rd[:])
                for ct in range(2):
                    transpose128(attnT[ct][:, ntile * 128:(ntile + 1) * 128],
                                 an[:, ct * 128:(ct + 1) * 128], psTC)
            hf = [hidC_pool.tile([128, NS], DT, tag=f"hidC{i}") for i in range(4)]
            mlp_l1_T(fw1, attnT, hf, NS, psC, fast_mlp)
            for ntile in range(NT_Q):
                o_ps = mlp_l2_nat(hf, fw2, ntile, psC, fast_mlp)
                o_sb = fin_pool.tile([128, D], DT, tag="osb")
                nc.vector.tensor_copy(o_sb[:], o_ps[:])
                nc.sync.dma_start(
                    out_sh.ap()[ntile * 128:(ntile + 1) * 128, :], o_sb[:])

    nc.finalize()
    return nc


_NC_CACHE = {}


def kernel(central_features, context_features, qw1, qw2, kw1, kw2,
           vw1, vw2, fw1, fw2, _trace=False, _return_results=False,
           _fast_mm=False, _fast_mlp=False):
    key = (_fast_mm, _fast_mlp)
    if key not in _NC_CACHE:
        _NC_CACHE[key] = build_nc(fast_mm=_fast_mm, fast_mlp=_fast_mlp)
    nc = _NC_CACHE[key]

    weights = {"qw1": qw1, "qw2": qw2, "kw1": kw1, "kw2": kw2,
               "vw1": vw1, "vw2": np.asarray(vw2, np.float32) / float(SS),
               "fw1": fw1, "fw2": fw2}
    weights = {k: np.ascontiguousarray(v, dtype=np.float32)
               for k, v in weights.items()}
    central_features = np.ascontiguousarray(central_features, dtype=np.float32)
    context_features = np.ascontiguousarray(context_features, dtype=np.float32)

    in_maps = []
    for r in range(M):
        m = {"central_sh": central_features[r * NS:(r + 1) * NS],
             "context_sh": context_features[r * TS:(r + 1) * TS]}
        m.update(weights)
        in_maps.append(m)

    res = run_bass_kernel_spmd(nc, in_maps, core_ids=list(range(M)),
                               trace=_trace)
    out = np.concatenate([res.results[r]["out_sh"] for r in range(M)], axis=0)
    if _return_results:
        return out, res
    return out


if __name__ == "__main__":
    rng = np.random.default_rng(0)
    f = lambda *s: rng.standard_normal(s, dtype=np.float32)
    ins = dict(central_features=f(N, C, S, S), context_features=f(T, C, S, S),
               qw1=f(C, H) * 0.02, qw2=f(H, D) * 0.02,
               kw1=f(C, H) * 0.02, kw2=f(H, D) * 0.02,
               vw1=f(C, H) * 0.02, vw2=f(H, D) * 0.02,
               fw1=f(D, H) * 0.02, fw2=f(H, C) * 0.02)
    out = kernel(**ins)
    print(out.shape, out.dtype, np.abs(out).max())


# revision 8
# speedup vs baseline: 1.5607x; 1.5607x over previous
# ContextRCNN attention-bias kernel for 8 Trainium2 NeuronCores.
#
# Reference computation (see problem statement):
#   central:[N,C,7,7] -> mean-pool -> Q-MLP -> l2norm -> queries [N,QK]
#   context:[T,C,7,7] -> mean-pool -> K/V-MLPs (K l2normed)
#   softmax(Q @ K^T * 6.25) @ V -> final MLP -> [N, C]
#
# Distribution (one SPMD NEFF on 8 cores, rank == q-shard == t-shard index):
#   - central rows sharded N/8=512 per core; context rows sharded T/8=1536.
#   - Each core pools+MLPs only its own shards (so the 822MB of input is read
#     exactly once across the chip), computes local queries, AllGathers the
#     (small, 4MB) query matrix, then computes the partial attention of ALL
#     4096 queries against its LOCAL 1536 keys/values:  numer_r = exp(S_r)^T
#     @ [V_r | 1].  A ReduceScatter sums the partials over cores and hands
#     rank r exactly its q-shard rows; divide by the gathered denominator
#     column and run the final MLP locally.
#   - Softmax needs no max-subtraction: logits are 6.25*cosine in [-6.25,6.25].
#
# Layout trick: "transposed" first MLP layers (lhsT = weight in its natural
# [in,out] layout, rhs = pooled^T) followed by natural second layers flip the
# orientation back, so no transposes are needed on the big T pipeline; only
# the small pooled/Q/attn matrices get PE transposes.

import numpy as np
from contextlib import ExitStack

import concourse.bass as bass
import concourse.mybir as mybir
import concourse.tile as tile
from concourse import bacc
from concourse.bass_utils import run_bass_kernel_spmd
from concourse.masks import make_identity

AF = mybir.ActivationFunctionType
DT = mybir.dt.float32

M = 8                    # cores
N, T, C, S = 4096, 12288, 256, 7
NS, TS = N // M, T // M  # 512 q rows / 1536 kv rows per core
H = 512                  # MLP hidden
D = 256                  # QK == VD == C
SS = S * S               # 49
SCALE = 1.0 / (0.01 * C ** 0.5)   # 6.25
NT_Q = NS // 128         # 4  q-tiles per core
NT_T = TS // 128         # 12 t-tiles per core
NCOL = D + 2             # V plus ones cols (padded even for fp32r)
CSPLIT = 4               # channel split for the raw pooling loads
CCH = C // CSPLIT        # channels per load
RAWF = CCH * SS          # floats per partition row per load

# float32r matmul operands stream at 1 cycle/row (4x faster than float32)
# with a reduced-precision multiply. The BIR verifier requires every SBUF
# tensor consumed by an fp32r matmul to be *written* as float32r (so values
# are pre-rounded), so the operand tiles/DRAM tensors themselves are declared
# float32r; numpy still sees plain fp32 bytes.


def build_nc(fast_mm=False, fast_mlp=False):
    nc = bacc.Bacc("TRN2", target_bir_lowering=False, debug=False, num_devices=M)
    adt = mybir.dt.float32r if fast_mm else DT    # attention matmul operands
    mdt = mybir.dt.float32r if fast_mlp else DT   # MLP matmul operands

    central = nc.dram_tensor("central_sh", [NS, C, S, S], DT, kind="ExternalInput")
    context = nc.dram_tensor("context_sh", [TS, C, S, S], DT, kind="ExternalInput")
    wnames = ["qw1", "qw2", "kw1", "kw2", "vw1", "vw2", "fw1", "fw2"]
    wshapes = {"1": [C, H], "2": [H, D]}
    wdram = {n: nc.dram_tensor(n, wshapes[n[-1]], mdt, kind="ExternalInput")
             for n in wnames}
    out_sh = nc.dram_tensor("out_sh", [NS, C], DT, kind="ExternalOutput")

    qt_in = nc.dram_tensor("qt_in", [D, NS], adt)                      # AG input
    qt_out = nc.dram_tensor("qt_out", [M * D, NS], adt, addr_space="Shared")
    rs_in = nc.dram_tensor("rs_in", [N, NCOL], DT)                     # RS input
    rs_out = nc.dram_tensor("rs_out", [NS, NCOL], DT)

    with tile.TileContext(nc) as tc, ExitStack() as ctx:
        ident_pool = ctx.enter_context(tc.tile_pool(name="ident", bufs=1))
        ident = ident_pool.tile([128, 128], DT)
        make_identity(nc, ident[:])

        # SBUF tensors that live across phases
        kvq_pool = ctx.enter_context(tc.tile_pool(name="kvq", bufs=1))
        ones_col = kvq_pool.tile([128, 2], DT, tag="ones", name="ones")
        nc.gpsimd.memset(ones_col[:], 1.0)
        kt_sb = [kvq_pool.tile([128, TS], DT, tag=f"kt{i}") for i in range(2)]
        vo_sb = [kvq_pool.tile([128, NCOL], DT, tag=f"vo{i}") for i in range(NT_T)]
        qt_all = [kvq_pool.tile([128, NS], DT, tag=f"qta{i}") for i in range(2 * M)]

        def transpose128(dst_sb, src_sb, tp_pool):
            """dst[128,128] = src[128,128]^T via PE."""
            ps = tp_pool.tile([128, 128], DT)
            nc.tensor.transpose(ps[:], src_sb, ident[:])
            nc.vector.tensor_copy(dst_sb, ps[:])

        def load_w(pool, name):
            """Load a weight matrix as a list of 128-row SBUF tiles."""
            shape = wshapes[name[-1]]
            tiles = []
            for i in range(shape[0] // 128):
                t = pool.tile([128, shape[1]], DT, tag=f"{name}_{i}")
                nc.sync.dma_start(t[:], wdram[name].ap()[i * 128:(i + 1) * 128, :])
                tiles.append(t)
            return tiles

        def pool_rows(dram_t, row0, raw_pool, pooled_pool):
            """Sum-pool 128 rows of [rows,C,7,7] -> pooled [128, C] tile.
            (The 1/49 mean scale cancels in l2norm for Q/K and is folded
            into vw2 on the host for V.)"""
            pooled = pooled_pool.tile([128, C], DT)
            src = dram_t.ap().rearrange("t c h w -> t (c h w)")
            for cs in range(CSPLIT):
                raw = raw_pool.tile([128, RAWF], DT, tag="raw")
                nc.sync.dma_start(
                    raw[:],
                    src[row0:row0 + 128, cs * RAWF:(cs + 1) * RAWF])
                nc.vector.reduce_sum(
                    pooled[:, cs * CCH:(cs + 1) * CCH],
                    raw[:].rearrange("p (c s) -> p c s", s=SS),
                    axis=mybir.AxisListType.X)
            return pooled

        def mlp_l1_T(w1_tiles, xT, out_tiles, nfree, ps_pool, fast):
            """hidden^T[h,n] = relu(w1^T @ x^T): out_tiles = 4 x [128, nfree]."""
            for ht in range(H // 128):
                for ch in range(nfree // 512):
                    ps = ps_pool.tile([128, 512], DT, tag="mm512")
                    for ck in range(C // 128):
                        nc.tensor.matmul(
                            ps[:],
                            w1_tiles[ck][:, ht * 128:(ht + 1) * 128],
                            xT[ck][:, ch * 512:(ch + 1) * 512],
                            start=(ck == 0), stop=(ck == 1))
                    nc.scalar.activation(
                        out_tiles[ht][:, ch * 512:(ch + 1) * 512], ps[:], AF.Relu)

        def mlp_l2_nat(hid_tiles, w2_tiles, nt, ps_pool, fast):
            """x[n,d] psum tile = hidden @ w2 for 128-row block nt."""
            ps = ps_pool.tile([128, D], DT, tag="mm256")
            for hk in range(H // 128):
                nc.tensor.matmul(
                    ps[:],
                    hid_tiles[hk][:, nt * 128:(nt + 1) * 128],
                    w2_tiles[hk][:],
                    start=(hk == 0), stop=(hk == 3))
            return ps

        # ---------------- Phase A: pooling, Q path, K/V path ----------------
        with tc.tile_pool(name="raw", bufs=2) as raw_pool, \
             tc.tile_pool(name="pooled", bufs=2) as pooled_pool, \
             tc.tile_pool(name="pooledT", bufs=1) as pt_pool, \
             tc.tile_pool(name="wA", bufs=1) as wA_pool, \
             tc.tile_pool(name="hid", bufs=1) as hid_pool, \
             tc.tile_pool(name="small", bufs=2) as small_pool, \
             tc.tile_pool(name="psA", bufs=2, space="PSUM") as psA, \
             tc.tile_pool(name="psT", bufs=2, space="PSUM") as psT:

            qw1 = load_w(wA_pool, "qw1"); qw2 = load_w(wA_pool, "qw2")
            kw1 = load_w(wA_pool, "kw1"); kw2 = load_w(wA_pool, "kw2")
            vw1 = load_w(wA_pool, "vw1"); vw2 = load_w(wA_pool, "vw2")

            # --- central: pool + transpose ---
            cenT = [pt_pool.tile([128, NS], DT, tag=f"cenT{i}") for i in range(2)]
            for ntile in range(NT_Q):
                pooled = pool_rows(central, ntile * 128, raw_pool, pooled_pool)
                for ct in range(2):
                    transpose128(cenT[ct][:, ntile * 128:(ntile + 1) * 128],
                                 pooled[:, ct * 128:(ct + 1) * 128], psT)

            # --- Q MLP (transposed l1, natural l2), l2norm * 6.25, transpose ---
            hq = [hid_pool.tile([128, NS], DT, tag=f"hid{i}") for i in range(4)]
            mlp_l1_T(qw1, cenT, hq, NS, psA, fast_mlp)
            qt_sh = [wA_pool.tile([128, NS], DT, tag=f"qtsh{i}") for i in range(2)]
            for ntile in range(NT_Q):
                q_ps = mlp_l2_nat(hq, qw2, ntile, psA, fast_mlp)
                sq = small_pool.tile([128, D], DT, tag="sq")
                nc.scalar.activation(sq[:], q_ps[:], AF.Square)
                ssq = small_pool.tile([128, 1], DT, tag="ssq")
                nc.vector.reduce_sum(ssq[:], sq[:], axis=mybir.AxisListType.X)
                nrm = small_pool.tile([128, 1], DT, tag="nrm")
                nc.scalar.activation(nrm[:], ssq[:], AF.Sqrt)
                rcp = small_pool.tile([128, 1], DT, tag="rcp")
                nc.vector.reciprocal(rcp[:], nrm[:])
                qn = small_pool.tile([128, D], DT, tag="qn")
                nc.vector.tensor_scalar(
                    qn[:], q_ps[:], rcp[:], SCALE,
                    op0=mybir.AluOpType.mult, op1=mybir.AluOpType.mult)
                for ct in range(2):
                    transpose128(qt_sh[ct][:, ntile * 128:(ntile + 1) * 128],
                                 qn[:, ct * 128:(ct + 1) * 128], psT)
            for ct in range(2):
                nc.sync.dma_start(qt_in.ap()[ct * 128:(ct + 1) * 128, :], qt_sh[ct][:])
            nc.gpsimd.collective_compute(
                "AllGather", mybir.AluOpType.bypass,
                replica_groups=[list(range(M))],
                ins=[qt_in[:]], outs=[qt_out[:]])

            # --- context: pool + transpose ---
            ctxT = [pt_pool.tile([128, TS], DT, tag=f"ctxT{i}") for i in range(2)]
            for ttile in range(NT_T):
                pooled = pool_rows(context, ttile * 128, raw_pool, pooled_pool)
                for ct in range(2):
                    transpose128(ctxT[ct][:, ttile * 128:(ttile + 1) * 128],
                                 pooled[:, ct * 128:(ct + 1) * 128], psT)

            # --- K path: transposed l1, natural l2, l2norm, transpose -> KT ---
            hk = [hid_pool.tile([128, TS], DT, tag=f"hid{i}") for i in range(4)]
            mlp_l1_T(kw1, ctxT, hk, TS, psA, fast_mlp)
            for ttile in range(NT_T):
                k_ps = mlp_l2_nat(hk, kw2, ttile, psA, fast_mlp)
                sq = small_pool.tile([128, D], DT, tag="sq")
                nc.scalar.activation(sq[:], k_ps[:], AF.Square)
                ssq = small_pool.tile([128, 1], DT, tag="ssq")
                nc.vector.reduce_sum(ssq[:], sq[:], axis=mybir.AxisListType.X)
                nrm = small_pool.tile([128, 1], DT, tag="nrm")
                nc.scalar.activation(nrm[:], ssq[:], AF.Sqrt)
                rcp = small_pool.tile([128, 1], DT, tag="rcp")
                nc.vector.reciprocal(rcp[:], nrm[:])
                kn = small_pool.tile([128, D], DT, tag="qn")
                nc.vector.tensor_scalar_mul(kn[:], k_ps[:], rcp[:])
                for ct in range(2):
                    transpose128(kt_sb[ct][:, ttile * 128:(ttile + 1) * 128],
                                 kn[:, ct * 128:(ct + 1) * 128], psT)

            # --- V path: transposed l1, natural l2 -> [V | 1] tiles ---
            hv = [hid_pool.tile([128, TS], DT, tag=f"hid{i}") for i in range(4)]
            mlp_l1_T(vw1, ctxT, hv, TS, psA, fast_mlp)
            for ttile in range(NT_T):
                v_ps = mlp_l2_nat(hv, vw2, ttile, psA, fast_mlp)
                nc.vector.tensor_copy(vo_sb[ttile][:, 0:D], v_ps[:])
                nc.vector.tensor_copy(vo_sb[ttile][:, D:NCOL], ones_col[:])

        # load gathered queries to SBUF
        for i in range(2 * M):
            nc.sync.dma_start(qt_all[i][:], qt_out.ap()[i * 128:(i + 1) * 128, :])

        # ---------------- Phase B: attention over local t ----------------
        with tc.tile_pool(name="est", bufs=3) as e_pool, \
             tc.tile_pool(name="nmcp", bufs=4) as nm_pool, \
             tc.tile_pool(name="psST", bufs=2, space="PSUM") as ps_st, \
             tc.tile_pool(name="psNM", bufs=4, space="PSUM") as ps_nm:
            for qb in range(M):
                nm_ps = [ps_nm.tile([128, NCOL], DT, tag="nm") for _ in range(NT_Q)]
                prev_e = None
                for tt in range(NT_T):
                    st = ps_st.tile([128, 512], DT, tag="st")
                    for dk in range(2):
                        nc.tensor.matmul(
                            st[:],
                            kt_sb[dk][:, tt * 128:(tt + 1) * 128],
                            qt_all[2 * qb + dk][:],
                            start=(dk == 0), stop=(dk == 1))
                    e_sb = e_pool.tile([128, 512], adt, tag="e")
                    nc.scalar.activation(e_sb[:], st[:], AF.Exp)
                    if prev_e is not None:
                        ptt = tt - 1
                        for qt in range(NT_Q):
                            nc.tensor.matmul(
                                nm_ps[qt][:],
                                prev_e[:, qt * 128:(qt + 1) * 128],
                                vo_sb[ptt][:],
                                start=(ptt == 0), stop=False,
                                skip_group_check=True)
                    prev_e = e_sb
                for qt in range(NT_Q):
                    nc.tensor.matmul(
                        nm_ps[qt][:],
                        prev_e[:, qt * 128:(qt + 1) * 128],
                        vo_sb[NT_T - 1][:],
                        start=False, stop=True, skip_group_check=True)
                for qt in range(NT_Q):
                    nm_sb = nm_pool.tile([128, NCOL], DT, tag="nmsb")
                    nc.vector.tensor_copy(nm_sb[:], nm_ps[qt][:])
                    r0 = qb * NS + qt * 128
                    nc.sync.dma_start(rs_in.ap()[r0:r0 + 128, :], nm_sb[:])

        nc.gpsimd.collective_compute(
            "ReduceScatter", mybir.AluOpType.add,
            replica_groups=[list(range(M))],
            ins=[rs_in[:]], outs=[rs_out[:]])

        # ---------------- Phase C: divide + final MLP ----------------
        with tc.tile_pool(name="fin", bufs=2) as fin_pool, \
             tc.tile_pool(name="wC", bufs=1) as wC_pool, \
             tc.tile_pool(name="hidC", bufs=4) as hidC_pool, \
             tc.tile_pool(name="psC", bufs=2, space="PSUM") as psC, \
             tc.tile_pool(name="psTC", bufs=2, space="PSUM") as psTC:
            fw1 = load_w(wC_pool, "fw1"); fw2 = load_w(wC_pool, "fw2")
            attnT = [fin_pool.tile([128, NS], DT, tag=f"attnT{i}") for i in range(2)]
            for ntile in range(NT_Q):
                att = fin_pool.tile([128, NCOL], DT, tag="att")
                nc.sync.dma_start(
                    att[:], rs_out.ap()[ntile * 128:(ntile + 1) * 128, :])
                rd = fin_pool.tile([128, 1], DT, tag="rd")
                nc.vector.reciprocal(rd[:], att[:, D:D + 1])
                an = fin_pool.tile([128, D], DT, tag="an")
                nc.vector.tensor_scalar_mul(an[:], att[:, 0:D], rd[:])
                for ct in range(2):
                    transpose128(attnT[ct][:, ntile * 128:(ntile + 1) * 128],
                                 an[:, ct * 128:(ct + 1) * 128], psTC)
            hf = [hidC_pool.tile([128, NS], DT, tag=f"hidC{i}") for i in range(4)]
            mlp_l1_T(fw1, attnT, hf, NS, psC, fast_mlp)
            for ntile in range(NT_Q):
                o_ps = mlp_l2_nat(hf, fw2, ntile, psC, fast_mlp)
                o_sb = fin_pool.tile([128, D], DT, tag="osb")
                nc.vector.tensor_copy(o_sb[:], o_ps[:])
                nc.sync.dma_start(
                    out_sh.ap()[ntile * 128:(ntile + 1) * 128, :], o_sb[:])

    nc.finalize()
    return nc


_NC_CACHE = {}


def kernel(central_features, context_features, qw1, qw2, kw1, kw2,
           vw1, vw2, fw1, fw2, _trace=False, _return_results=False,
           _fast_mm=False, _fast_mlp=False):
    key = (_fast_mm, _fast_mlp)
    if key not in _NC_CACHE:
        _NC_CACHE[key] = build_nc(fast_mm=_fast_mm, fast_mlp=_fast_mlp)
    nc = _NC_CACHE[key]

    weights = {"qw1": qw1, "qw2": qw2, "kw1": kw1, "kw2": kw2,
               "vw1": vw1, "vw2": np.asarray(vw2, np.float32) / float(SS),
               "fw1": fw1, "fw2": fw2}
    weights = {k: np.ascontiguousarray(v, dtype=np.float32)
               for k, v in weights.items()}
    central_features = np.ascontiguousarray(central_features, dtype=np.float32)
    context_features = np.ascontiguousarray(context_features, dtype=np.float32)

    in_maps = []
    for r in range(M):
        m = {"central_sh": central_features[r * NS:(r + 1) * NS],
             "context_sh": context_features[r * TS:(r + 1) * TS]}
        m.update(weights)
        in_maps.append(m)

    res = run_bass_kernel_spmd(nc, in_maps, core_ids=list(range(M)),
                               trace=_trace)
    out = np.concatenate([res.results[r]["out_sh"] for r in range(M)], axis=0)
    if _return_results:
        return out, res
    return out


if __name__ == "__main__":
    rng = np.random.default_rng(0)
    f = lambda *s: rng.standard_normal(s, dtype=np.float32)
    ins = dict(central_features=f(N, C, S, S), context_features=f(T, C, S, S),
               qw1=f(C, H) * 0.02, qw2=f(H, D) * 0.02,
               kw1=f(C, H) * 0.02, kw2=f(H, D) * 0.02,
               vw1=f(C, H) * 0.02, vw2=f(H, D) * 0.02,
               fw1=f(D, H) * 0.02, fw2=f(H, C) * 0.02)
    out = kernel(**ins)
    print(out.shape, out.dtype, np.abs(out).max())


# revision 9
# speedup vs baseline: 1.6262x; 1.0420x over previous
# ContextRCNN attention-bias kernel for 8 Trainium2 NeuronCores.
#
# Reference computation (see problem statement):
#   central:[N,C,7,7] -> mean-pool -> Q-MLP -> l2norm -> queries [N,QK]
#   context:[T,C,7,7] -> mean-pool -> K/V-MLPs (K l2normed)
#   softmax(Q @ K^T * 6.25) @ V -> final MLP -> [N, C]
#
# Distribution (one SPMD NEFF on 8 cores, rank == q-shard == t-shard index):
#   - central rows sharded N/8=512 per core; context rows sharded T/8=1536,
#     so the 822MB of input is read exactly once across the chip (~103MB/core,
#     the memory roofline for this problem).
#   - Each core pools+MLPs its own shards, computes local queries, AllGathers
#     the (small, 4MB) query matrix, then computes the partial attention of
#     ALL 4096 queries against its LOCAL keys/values: numer_r = exp(S_r)^T @
#     [V_r | 1]. A ReduceScatter sums the partials over cores and hands rank
#     r exactly its q-shard rows; divide by the gathered denominator column
#     and run the final MLP locally.
#   - Softmax needs no max-subtraction: logits are 6.25*cosine in [-6.25,6.25].
#   - The context stream is processed in chunks of 4 row-tiles; each chunk's
#     K/V MLP + attention runs while the next chunk DMAs, so PE/ACT work
#     hides under the ~300 GB/s input stream instead of serializing after it.
#
# Layout: "transposed" first MLP layers (lhsT = weight in its natural
# [in,out] layout, rhs = pooled^T) followed by natural second layers flip the
# orientation back, so no transposes are needed on the big T pipeline; only
# the small pooled/Q/attn matrices get PE transposes.
#
# Precision: the two big attention matmuls (scores, exp@V) use float32r
# operands (full-rate PE streaming). Host emulation shows rounding only the
# attention operands (l2-normalized Q/K, softmax weights, V) costs ~1.4e-5
# relative error, while fp32r MLPs would amplify to ~5e-4 — so the MLPs stay
# full fp32 (they fit under the DMA-bound phase anyway).

import numpy as np
from contextlib import ExitStack

import concourse.bass as bass
import concourse.mybir as mybir
import concourse.tile as tile
from concourse import bacc
from concourse.bass_utils import run_bass_kernel_spmd
from concourse.masks import make_identity

AF = mybir.ActivationFunctionType
DT = mybir.dt.float32

M = 8                    # cores
N, T, C, S = 4096, 12288, 256, 7
NS, TS = N // M, T // M  # 512 q rows / 1536 kv rows per core
H = 512                  # MLP hidden
D = 256                  # QK == VD == C
SS = S * S               # 49
SCALE = 1.0 / (0.01 * C ** 0.5)   # 6.25
NT_Q = NS // 128         # 4  q-tiles per core
NT_T = TS // 128         # 12 t-tiles per core
NCOL = D + 2             # V plus ones cols (padded even for fp32r)
CSPLIT = 4               # channel split for the raw pooling loads
CCH = C // CSPLIT        # channels per load
RAWF = CCH * SS          # floats per partition row per load
TCHUNK = 4               # t-tiles per context chunk
NCHUNK = NT_T // TCHUNK  # 3


def build_nc(fast_mm=True, fast_mlp=False):
    nc = bacc.Bacc("TRN2", target_bir_lowering=False, debug=False, num_devices=M)
    adt = mybir.dt.float32r if fast_mm else DT    # attention matmul operands
    mdt = mybir.dt.float32r if fast_mlp else DT   # MLP matmul operands

    central = nc.dram_tensor("central_sh", [NS, C, S, S], DT, kind="ExternalInput")
    context = nc.dram_tensor("context_sh", [TS, C, S, S], DT, kind="ExternalInput")
    wnames = ["qw1", "qw2", "kw1", "kw2", "vw1", "vw2", "fw1", "fw2"]
    wshapes = {"1": [C, H], "2": [H, D]}
    wdram = {n: nc.dram_tensor(n, wshapes[n[-1]], mdt, kind="ExternalInput")
             for n in wnames}
    out_sh = nc.dram_tensor("out_sh", [NS, C], DT, kind="ExternalOutput")

    qt_in = nc.dram_tensor("qt_in", [D, NS], adt)                      # AG input
    qt_out = nc.dram_tensor("qt_out", [M * D, NS], adt, addr_space="Shared")
    rs_in = nc.dram_tensor("rs_in", [N, NCOL], DT)                     # RS input
    rs_out = nc.dram_tensor("rs_out", [NS, NCOL], DT)

    with tile.TileContext(nc) as tc, ExitStack() as ctx:
        ident_pool = ctx.enter_context(tc.tile_pool(name="ident", bufs=1))
        ident = ident_pool.tile([128, 128], DT)
        make_identity(nc, ident[:])

        # SBUF tensors that live across phases
        kvq_pool = ctx.enter_context(tc.tile_pool(name="kvq", bufs=1))
        ones_col = kvq_pool.tile([128, 2], DT, tag="ones", name="ones")
        nc.gpsimd.memset(ones_col[:], 1.0)
        kt_sb = [kvq_pool.tile([128, TS], adt, tag=f"kt{i}", name=f"kt{i}")
                 for i in range(2)]
        vo_sb = [kvq_pool.tile([128, NCOL], adt, tag=f"vo{i}", name=f"vo{i}")
                 for i in range(NT_T)]
        qt_all = [kvq_pool.tile([128, NS], adt, tag=f"qta{i}", name=f"qta{i}")
                  for i in range(2 * M)]
        # numerator accumulators for all 32 q-tiles (summed over chunks)
        nm_pool = ctx.enter_context(tc.tile_pool(name="nm", bufs=1))
        nm_sb = [nm_pool.tile([128, NCOL], DT, tag=f"nm{i}", name=f"nm{i}")
                 for i in range(N // 128)]

        def transpose128(dst_sb, src_sb, tp_pool, eng="vector"):
            """dst[128,128] = src[128,128]^T via PE (copy-out on eng)."""
            ps = tp_pool.tile([128, 256], DT, tag="ps_small", name="tp")
            nc.tensor.transpose(ps[:, 0:128], src_sb, ident[:])
            if eng == "scalar":
                nc.scalar.copy(dst_sb, ps[:, 0:128])
            else:
                nc.vector.tensor_copy(dst_sb, ps[:, 0:128])

        def load_w(pool, name):
            shape = wshapes[name[-1]]
            tiles = []
            for i in range(shape[0] // 128):
                t = pool.tile([128, shape[1]], mdt, tag=f"{name}_{i}",
                              name=f"{name}_{i}")
                nc.sync.dma_start(t[:], wdram[name].ap()[i * 128:(i + 1) * 128, :])
                tiles.append(t)
            return tiles

        def pool_rows(dram_t, row0, raw_pool, pooled_pool):
            """Sum-pool 128 rows of [rows,C,7,7] -> pooled [128, C] tile.
            (The 1/49 mean scale cancels in l2norm for Q/K and is folded
            into vw2 on the host for V.)"""
            pooled = pooled_pool.tile([128, C], DT)
            src = dram_t.ap().rearrange("t c h w -> t (c h w)")
            for cs in range(CSPLIT):
                raw = raw_pool.tile([128, RAWF], DT, tag="raw")
                nc.sync.dma_start(
                    raw[:], src[row0:row0 + 128, cs * RAWF:(cs + 1) * RAWF])
                nc.vector.reduce_sum(
                    pooled[:, cs * CCH:(cs + 1) * CCH],
                    raw[:].rearrange("p (c s) -> p c s", s=SS),
                    axis=mybir.AxisListType.X)
            return pooled

        def mlp_l1_T(w1_tiles, xT, xcol0, out_tiles, nfree, ps_pool):
            """hidden^T[h,n] = relu(w1^T @ x^T) over a 512-wide column block."""
            for ht in range(H // 128):
                for ch in range(nfree // 512):
                    ps = ps_pool.tile([128, 512], DT, tag="mm512")
                    for ck in range(C // 128):
                        nc.tensor.matmul(
                            ps[:],
                            w1_tiles[ck][:, ht * 128:(ht + 1) * 128],
                            xT[ck][:, xcol0 + ch * 512:xcol0 + (ch + 1) * 512],
                            start=(ck == 0), stop=(ck == 1))
                    nc.scalar.activation(
                        out_tiles[ht][:, ch * 512:(ch + 1) * 512], ps[:], AF.Relu)

        def mlp_l2_nat(hid_tiles, w2_tiles, nt, ps_pool):
            """x[n,d] psum tile = hidden @ w2 for 128-row block nt."""
            ps = ps_pool.tile([128, 256], DT, tag="ps_small", name="l2ps")
            for hk in range(H // 128):
                nc.tensor.matmul(
                    ps[:],
                    hid_tiles[hk][:, nt * 128:(nt + 1) * 128],
                    w2_tiles[hk][:],
                    start=(hk == 0), stop=(hk == 3))
            return ps

        def l2norm_recip(src_ps, pool, extra_scale=None):
            """1/||row|| (optionally *extra_scale) as [128,1] from psum tile."""
            sq = pool.tile([128, D], DT, tag="sq", name="sq")
            nc.scalar.activation(sq[:], src_ps[:], AF.Square)
            ssq = pool.tile([128, 1], DT, tag="ssq", name="ssq")
            nc.vector.reduce_sum(ssq[:], sq[:], axis=mybir.AxisListType.X)
            nrm = pool.tile([128, 1], DT, tag="nrm", name="nrm")
            nc.scalar.activation(nrm[:], ssq[:], AF.Sqrt)
            rcp = pool.tile([128, 1], DT, tag="rcp", name="rcp")
            nc.vector.reciprocal(rcp[:], nrm[:])
            return rcp

        with tc.tile_pool(name="raw", bufs=2) as raw_pool, \
             tc.tile_pool(name="pooled", bufs=2) as pooled_pool, \
             tc.tile_pool(name="ptq", bufs=1) as ptq_pool, \
             tc.tile_pool(name="ptc", bufs=2) as ptc_pool, \
             tc.tile_pool(name="wA", bufs=1) as wA_pool, \
             tc.tile_pool(name="hid", bufs=1) as hid_pool, \
             tc.tile_pool(name="small", bufs=2) as small_pool, \
             tc.tile_pool(name="est", bufs=6) as e_pool, \
             tc.tile_pool(name="ps512", bufs=2, space="PSUM") as ps512, \
             tc.tile_pool(name="psSm", bufs=2, space="PSUM") as psSm, \
             tc.tile_pool(name="psST", bufs=2, space="PSUM") as ps_st, \
             tc.tile_pool(name="psNM", bufs=2, space="PSUM") as ps_nm:

            qw1 = load_w(wA_pool, "qw1"); qw2 = load_w(wA_pool, "qw2")
            kw1 = load_w(wA_pool, "kw1"); kw2 = load_w(wA_pool, "kw2")
            vw1 = load_w(wA_pool, "vw1"); vw2 = load_w(wA_pool, "vw2")

            # --- central: pool + transpose -> cenT [c, NS] ---
            cenT = [ptq_pool.tile([128, NS], mdt, tag=f"cenT{i}", name=f"cenT{i}")
                    for i in range(2)]
            for ntile in range(NT_Q):
                pooled = pool_rows(central, ntile * 128, raw_pool, pooled_pool)
                for ct in range(2):
                    transpose128(cenT[ct][:, ntile * 128:(ntile + 1) * 128],
                                 pooled[:, ct * 128:(ct + 1) * 128], psSm)

            # --- Q MLP, l2norm * 6.25, transpose, AllGather ---
            hq = [hid_pool.tile([128, NS], mdt, tag=f"hid{i}", name=f"hq{i}")
                  for i in range(4)]
            mlp_l1_T(qw1, cenT, 0, hq, NS, ps512)
            qt_sh = [wA_pool.tile([128, NS], adt, tag=f"qtsh{i}", name=f"qtsh{i}")
                     for i in range(2)]
            for ntile in range(NT_Q):
                q_ps = mlp_l2_nat(hq, qw2, ntile, psSm)
                rcp = l2norm_recip(q_ps, small_pool)
                qn = small_pool.tile([128, D], DT, tag="qn", name="qn")
                nc.vector.tensor_scalar(
                    qn[:], q_ps[:], rcp[:], SCALE,
                    op0=mybir.AluOpType.mult, op1=mybir.AluOpType.mult)
                for ct in range(2):
                    transpose128(qt_sh[ct][:, ntile * 128:(ntile + 1) * 128],
                                 qn[:, ct * 128:(ct + 1) * 128], psSm)
            for ct in range(2):
                nc.sync.dma_start(qt_in.ap()[ct * 128:(ct + 1) * 128, :],
                                  qt_sh[ct][:])
            nc.gpsimd.collective_compute(
                "AllGather", mybir.AluOpType.bypass,
                replica_groups=[list(range(M))],
                ins=[qt_in[:]], outs=[qt_out[:]])
            for i in range(2 * M):
                nc.sync.dma_start(qt_all[i][:],
                                  qt_out.ap()[i * 128:(i + 1) * 128, :])

            # --- context chunks: pool -> K/V MLP -> attention, pipelined ---
            for chunk in range(NCHUNK):
                tt0 = chunk * TCHUNK
                ctxT = [ptc_pool.tile([128, TCHUNK * 128], mdt, tag=f"ctxT{i}",
                                      name=f"ctxT{chunk}_{i}") for i in range(2)]
                for j in range(TCHUNK):
                    pooled = pool_rows(context, (tt0 + j) * 128,
                                       raw_pool, pooled_pool)
                    for ct in range(2):
                        transpose128(ctxT[ct][:, j * 128:(j + 1) * 128],
                                     pooled[:, ct * 128:(ct + 1) * 128],
                                     psSm, eng="scalar")
                # K path for this chunk
                hk = [hid_pool.tile([128, TCHUNK * 128], mdt, tag=f"hid{i}",
                                    name=f"hk{chunk}_{i}") for i in range(4)]
                mlp_l1_T(kw1, ctxT, 0, hk, TCHUNK * 128, ps512)
                for j in range(TCHUNK):
                    k_ps = mlp_l2_nat(hk, kw2, j, psSm)
                    rcp = l2norm_recip(k_ps, small_pool)
                    kn = small_pool.tile([128, D], DT, tag="qn", name="kn")
                    nc.vector.tensor_scalar_mul(kn[:], k_ps[:], rcp[:])
                    for ct in range(2):
                        tcol = (tt0 + j) * 128
                        transpose128(kt_sb[ct][:, tcol:tcol + 128],
                                     kn[:, ct * 128:(ct + 1) * 128],
                                     psSm, eng="scalar")
                # V path for this chunk
                hv = [hid_pool.tile([128, TCHUNK * 128], mdt, tag=f"hid{i}",
                                    name=f"hv{chunk}_{i}") for i in range(4)]
                mlp_l1_T(vw1, ctxT, 0, hv, TCHUNK * 128, ps512)
                for j in range(TCHUNK):
                    v_ps = mlp_l2_nat(hv, vw2, j, psSm)
                    nc.vector.tensor_copy(vo_sb[tt0 + j][:, 0:D], v_ps[:])
                    nc.vector.tensor_copy(vo_sb[tt0 + j][:, D:NCOL], ones_col[:])

                # attention of ALL q-blocks against this chunk's K/V.
                # Emit q-block (qb) score+exp before the previous q-block's
                # numer matmuls so exp latency hides under PE work.
                prev_e = None

                def numer_for(e_tiles, qb):
                    for qt in range(NT_Q):
                        nm_ps = ps_nm.tile([128, NCOL], DT, tag="nmps",
                                           name="nmps")
                        for j in range(TCHUNK):
                            nc.tensor.matmul(
                                nm_ps[:],
                                e_tiles[j][:, qt * 128:(qt + 1) * 128],
                                vo_sb[tt0 + j][:],
                                start=(j == 0), stop=(j == TCHUNK - 1),
                                skip_group_check=True)
                        qrow = qb * NT_Q + qt
                        if chunk == 0:
                            nc.vector.tensor_copy(nm_sb[qrow][:], nm_ps[:])
                        else:
                            nc.vector.tensor_add(nm_sb[qrow][:],
                                                 nm_sb[qrow][:], nm_ps[:])

                for qb in range(M):
                    e_tiles = []
                    for j in range(TCHUNK):
                        st = ps_st.tile([128, 512], DT, tag="st")
                        for dk in range(2):
                            tcol = (tt0 + j) * 128
                            nc.tensor.matmul(
                                st[:], kt_sb[dk][:, tcol:tcol + 128],
                                qt_all[2 * qb + dk][:],
                                start=(dk == 0), stop=(dk == 1))
                        e_sb = e_pool.tile([128, 512], adt, tag="e", name="esb")
                        nc.scalar.activation(e_sb[:], st[:], AF.Exp)
                        e_tiles.append(e_sb)
                    if prev_e is not None:
                        numer_for(prev_e, qb - 1)
                    prev_e = e_tiles
                numer_for(prev_e, M - 1)

            for qrow in range(N // 128):
                nc.sync.dma_start(rs_in.ap()[qrow * 128:(qrow + 1) * 128, :],
                                  nm_sb[qrow][:])

        nc.gpsimd.collective_compute(
            "ReduceScatter", mybir.AluOpType.add,
            replica_groups=[list(range(M))],
            ins=[rs_in[:]], outs=[rs_out[:]])

        # ---------------- divide + final MLP ----------------
        with tc.tile_pool(name="fin", bufs=2) as fin_pool, \
             tc.tile_pool(name="wC", bufs=1) as wC_pool, \
             tc.tile_pool(name="hidC", bufs=1) as hidC_pool, \
             tc.tile_pool(name="psC", bufs=2, space="PSUM") as psC, \
             tc.tile_pool(name="psTC", bufs=2, space="PSUM") as psTC:
            fw1 = load_w(wC_pool, "fw1"); fw2 = load_w(wC_pool, "fw2")
            attnT = [fin_pool.tile([128, NS], mdt, tag=f"attnT{i}",
                                   name=f"attnT{i}") for i in range(2)]
            for ntile in range(NT_Q):
                att = fin_pool.tile([128, NCOL], DT, tag="att", name="att")
                nc.sync.dma_start(
                    att[:], rs_out.ap()[ntile * 128:(ntile + 1) * 128, :])
                rd = fin_pool.tile([128, 1], DT, tag="rd", name="rd")
                nc.vector.reciprocal(rd[:], att[:, D:D + 1])
                an = fin_pool.tile([128, D], DT, tag="an", name="an")
                nc.vector.tensor_scalar_mul(an[:], att[:, 0:D], rd[:])
                for ct in range(2):
                    transpose128(attnT[ct][:, ntile * 128:(ntile + 1) * 128],
                                 an[:, ct * 128:(ct + 1) * 128], psTC)
            hf = [hidC_pool.tile([128, NS], mdt, tag=f"hidC{i}", name=f"hfC{i}")
                  for i in range(4)]
            mlp_l1_T(fw1, attnT, 0, hf, NS, psC)
            for ntile in range(NT_Q):
                o_ps = psC.tile([128, 256], DT, tag="ops", name="ops")
                for hk_i in range(H // 128):
                    nc.tensor.matmul(
                        o_ps[:], hf[hk_i][:, ntile * 128:(ntile + 1) * 128],
                        fw2[hk_i][:], start=(hk_i == 0), stop=(hk_i == 3))
                o_sb = fin_pool.tile([128, D], DT, tag="osb", name="osb")
                nc.vector.tensor_copy(o_sb[:], o_ps[:])
                nc.sync.dma_start(
                    out_sh.ap()[ntile * 128:(ntile + 1) * 128, :], o_sb[:])

    nc.finalize()
    return nc


_NC_CACHE = {}


def kernel(central_features, context_features, qw1, qw2, kw1, kw2,
           vw1, vw2, fw1, fw2, _trace=False, _return_results=False,
           _fast_mm=True, _fast_mlp=False):
    key = (_fast_mm, _fast_mlp)
    if key not in _NC_CACHE:
        _NC_CACHE[key] = build_nc(fast_mm=_fast_mm, fast_mlp=_fast_mlp)
    nc = _NC_CACHE[key]

    weights = {"qw1": qw1, "qw2": qw2, "kw1": kw1, "kw2": kw2,
               "vw1": vw1, "vw2": np.asarray(vw2, np.float32) / float(SS),
               "fw1": fw1, "fw2": fw2}
    weights = {k: np.ascontiguousarray(v, dtype=np.float32)
               for k, v in weights.items()}
    central_features = np.ascontiguousarray(central_features, dtype=np.float32)
    context_features = np.ascontiguousarray(context_features, dtype=np.float32)

    in_maps = []
    for r in range(M):
        m = {"central_sh": central_features[r * NS:(r + 1) * NS],
             "context_sh": context_features[r * TS:(r + 1) * TS]}
        m.update(weights)
        in_maps.append(m)

    res = run_bass_kernel_spmd(nc, in_maps, core_ids=list(range(M)),
                               trace=_trace)
    out = np.concatenate([res.results[r]["out_sh"] for r in range(M)], axis=0)
    if _return_results:
        return out, res
    return out


if __name__ == "__main__":
    rng = np.random.default_rng(0)
    f = lambda *s: rng.standard_normal(s, dtype=np.float32)
    ins = dict(central_features=f(N, C, S, S), context_features=f(T, C, S, S),
               qw1=f(C, H) * 0.02, qw2=f(H, D) * 0.02,
               kw1=f(C, H) * 0.02, kw2=f(H, D) * 0.02,
               vw1=f(C, H) * 0.02, vw2=f(H, D) * 0.02,
               fw1=f(D, H) * 0.02, fw2=f(H, C) * 0.02)
    out = kernel(**ins)
    print(out.shape, out.dtype, np.abs(out).max())
